# revision 13
# baseline (speedup 1.0000x reference)
"""Causal self-attention (B=2, T=2048, C=1024, H=16, D=64) on 8 Trainium2
NeuronCores.

Sharding: core = (batch, head-group): b = core // 4, hg = core % 4; each core
computes 4 heads of one batch plus its partial out-projection (256 of 1024
contraction channels). Host sums the 4 partial y's per batch.

Per-core pipeline:
  1. qT/kT = (w_qk.T).T @ xT via fp8e4m3 DoubleRow matmuls (K=256 per
     instruction, 0.5 cycles/row): host pre-scales wq by 256/sqrt(D) and wk
     by 64 so the fp8 weights sit in-range; the 1/16384 descale is folded
     into the exp's scale operand. v = xT.T @ wv.T in bf16 (the value path
     cannot afford fp8 error).
  2. scores^T[tk, tq] = kT.T @ qT per head in bf16, block [128, 512], causal
     block-skip; diagonal blocks trimmed to the valid column range (bf16 has
     no >=256 moving-size requirement). exp on ACT (PSUM->SBUF, bf16 out,
     scale=1/16384); causal mask via gpsimd affine_select restricted to the
     single 128-col window per diagonal tile that av actually reads.
  3. outT[65, tq] = [v|1].T @ expS^T accumulated over tk tiles (bf16); row 64
     is the softmax denominator. Normalize: reciprocal + partition_broadcast
     + tensor_mul into pair-stacked attnout^T tiles [128, 512] bf16.
  4. y[t, o] += attnout^T.T @ w_out^T per head-pair (K=128, bf16), PSUM
     DMA'd directly to DRAM (no SBUF staging).
"""

import numpy as np

B, T, C = 2, 2048, 1024
H, D = 16, 64
N_CORES = 8
HG = 4  # head-groups (cores per batch)
HPC = 4  # heads per core
NCH = T // 512  # 4 tq chunks of 512
KT = T // 128  # 16 tk tiles of 128
CK = C // 128  # 8 contraction k-tiles for the bf16 projections
KK = C // 256  # 4 fp8 DoubleRow k-tiles (K=256 each) for the qk projection
SQ = 256.0  # wq prescale (includes 1/sqrt(D) fold)
SK = 64.0  # wk prescale
ESCL = 1.0 / (SQ * SK)  # exp descale

_NC = None


def _build_nc():
    import concourse.mybir as mybir
    import concourse.tile as tile
    from concourse import bacc

    F32 = mybir.dt.float32
    BF16 = mybir.dt.bfloat16
    F8 = mybir.dt.float8e4
    DR = mybir.MatmulPerfMode.DoubleRow

    nc = bacc.Bacc(None, target_bir_lowering=False)
    xT = nc.dram_tensor("xT", [128, CK, T], BF16, kind="ExternalInput")
    x8 = nc.dram_tensor("x8", [128, 2 * KK, T], F8, kind="ExternalInput")
    wqk8 = nc.dram_tensor("wqk8", [128, KK * 1024], F8, kind="ExternalInput")
    wv = nc.dram_tensor("wv", [128, CK, 256], BF16, kind="ExternalInput")
    wout = nc.dram_tensor("wout", [2, 128, C], BF16, kind="ExternalInput")
    ones = nc.dram_tensor("ones", [128, 4], BF16, kind="ExternalInput")
    y = nc.dram_tensor("y", [T, C], BF16, kind="ExternalOutput")

    with tile.TileContext(nc) as tc:
        with (
            tc.tile_pool(name="const", bufs=1) as const,
            tc.tile_pool(name="xin", bufs=3) as xin,
            tc.tile_pool(name="x8in", bufs=3) as x8in,
            tc.tile_pool(name="qk", bufs=1) as qkp,
            tc.tile_pool(name="vt", bufs=1) as vtp,
            tc.tile_pool(name="es", bufs=8) as esp,
            tc.tile_pool(name="ao", bufs=1) as aop,
            tc.tile_pool(name="nrm", bufs=3) as nrm,
            tc.tile_pool(name="yo", bufs=4) as yop,
            tc.tile_pool(name="mm", bufs=2, space="PSUM") as mmps,
            tc.tile_pool(name="sc", bufs=2, space="PSUM") as scps,
            tc.tile_pool(name="av", bufs=2, space="PSUM") as avps,
        ):
            # ---- weights + first x chunk. DMA queue order is the startup
            # critical path: qk weights + first x8 halves first (DoubleRow
            # proj can begin ~2us in), then wv + xt for the v projection;
            # wout is deferred to after proj chunk 1 (first needed ~30us in).
            wqk8_sb = const.tile([128, KK * 1024], F8, tag="wqk8")
            nc.sync.dma_start(wqk8_sb[:, 0 : 2 * 1024], wqk8[:, 0 : 2 * 1024])
            x8_first = x8in.tile([128, KK * 1024], F8, tag="x8", name="x8_0")
            nc.sync.dma_start(
                x8_first[:, 0:2048].rearrange("p (g t) -> p g t", g=4),
                x8[:, 0:4, 0:512],
            )
            nc.sync.dma_start(wqk8_sb[:, 2 * 1024 :], wqk8[:, 2 * 1024 :])
            nc.sync.dma_start(
                x8_first[:, 2048:].rearrange("p (g t) -> p g t", g=4),
                x8[:, 4:8, 0:512],
            )
            wv_all = const.tile([128, CK * 256], BF16, tag="wvall")
            nc.sync.dma_start(
                wv_all[:].rearrange("p (g c) -> p g c", g=CK), wv[:, :, :]
            )
            wv_t = [wv_all[:, k * 256 : (k + 1) * 256] for k in range(CK)]
            xt_first = []
            for k in range(CK):
                t_ = xin.tile([128, 512], BF16, tag=f"xt{k}", name=f"xt0_{k}", bufs=1)
                nc.sync.dma_start(t_[:], xT[:, k, 0:512])
                xt_first.append(t_)
            ones_sb = const.tile([128, 4], BF16, tag="ones")
            nc.sync.dma_start(ones_sb[:], ones[:])

            # wqk8 lhsT access for (kk, m): [128, 2, 128]
            def wqk8_ap(kk, m):
                return wqk8_sb[
                    :, kk * 1024 + m * 256 : kk * 1024 + (m + 1) * 256
                ].rearrange("p (i c) -> p i c", i=2)

            # ---- v tiles with ones columns ----
            v_t = []
            for t in range(KT):
                t_ = vtp.tile([128, 4 * 65], BF16, tag=f"v{t}")
                nc.vector.tensor_copy(
                    t_[:].rearrange("p (h e) -> p h e", h=4)[:, :, 64:65],
                    ones_sb[:].rearrange("p (h e) -> p h e", e=1),
                )
                v_t.append(t_)

            # ---- projections, streamed by tq/n chunk ----
            # qkT[m] tiles per (m, n): m 0..1 = q head-pairs, 2..3 = k pairs
            qkT = [[None] * NCH for _ in range(4)]

            def proj_chunk(n):
                if n == 0:
                    x8_n = x8_first
                    xt_n = xt_first
                else:
                    x8_n = x8in.tile(
                        [128, KK * 1024], F8, tag="x8", name=f"x8_{n}", bufs=3
                    )
                    nc.sync.dma_start(
                        x8_n[:].rearrange("p (g t) -> p g t", g=2 * KK),
                        x8[:, :, n * 512 : (n + 1) * 512],
                    )
                    xt_n = []
                    for half in range(2):
                        big = xin.tile(
                            [128, 4 * 512], BF16, tag=f"xth{half}",
                            name=f"xth{half}_{n}", bufs=2,
                        )
                        nc.sync.dma_start(
                            big[:].rearrange("p (g t) -> p g t", g=4),
                            xT[:, half * 4 : half * 4 + 4, n * 512 : (n + 1) * 512],
                        )
                        xt_n += [big[:, k * 512 : (k + 1) * 512] for k in range(4)]
                for m in range(4):
                    ps = mmps.tile([128, 512], F32, tag="mm")
                    for kk in range(KK):
                        nc.tensor.matmul(
                            ps[:],
                            wqk8_ap(kk, m),
                            x8_n[
                                :, kk * 1024 : (kk + 1) * 1024
                            ].rearrange("p (i t) -> p i t", i=2),
                            start=(kk == 0),
                            stop=(kk == KK - 1),
                            perf_mode=DR,
                        )
                    sb = qkp.tile([128, 512], BF16, tag=f"qk{m}_{n}", name=f"qk{m}_{n}")
                    nc.vector.tensor_copy(sb[:], ps[:])
                    qkT[m][n] = sb
                for ts in range(4):  # t-subtiles of this chunk
                    t = n * 4 + ts
                    ps = mmps.tile([128, 256], F32, tag="mm")
                    for k in range(CK):
                        nc.tensor.matmul(
                            ps[:],
                            xt_n[k][:, ts * 128 : (ts + 1) * 128],
                            wv_t[k][:],
                            start=(k == 0),
                            stop=(k == CK - 1),
                        )
                    nc.vector.tensor_copy(
                        v_t[t][:].rearrange("p (h e) -> p h e", h=4)[:, :, 0:64],
                        ps[:].rearrange("p (h d) -> p h d", h=4),
                    )

            # ---- attention + out-projection, per tq chunk ----
            # Blocks are software-pipelined: scores+exp run one block ahead of
            # the attn@v accumulation so PE never stalls on ACT. Off-diagonal
            # tk tiles are computed in pairs sharing one [128,1024] exp; the 4
            # diagonal tiles get exps/scores trimmed to the column range that
            # can be valid, and a 128-col affine_select causal window.
            attnout = [[None] * NCH for _ in range(2)]
            wout_t = []

            def emit_wout_load():
                for p in range(2):
                    t_ = const.tile([128, C], BF16, tag=f"wout{p}")
                    nc.sync.dma_start(t_[:], wout[p])
                    wout_t.append(t_)

            def emit_unit_blocks(h, q, pav):
                """Yields av-emit thunks, one per block, after emitting that
                block's scores+exp instructions."""
                p, s = h // 2, h % 2
                r0, r1 = 64 * s, 64 * s + 64
                vslice = slice(65 * h, 65 * h + 65)
                last_j = 4 * q + 3

                def sc_mm(out_ap, j, trim=0):
                    nc.tensor.matmul(
                        out_ap,
                        qkT[2 + p][j // 4][r0:r1, (j % 4) * 128 : (j % 4 + 1) * 128],
                        qkT[p][q][r0:r1, trim:512],
                        start=True,
                        stop=True,
                    )

                def av_mm(pav, es_ap, j, trim=0):
                    nc.tensor.matmul(
                        pav[:, trim:512],
                        v_t[j][:, vslice],
                        es_ap,
                        start=(j == 0),
                        stop=(j == last_j),
                    )

                for j0 in range(0, 4 * q, 2):  # off-diagonal pairs
                    psc = scps.tile([128, 1024], F32, tag="sc")
                    sc_mm(psc[:, 0:512], j0)
                    sc_mm(psc[:, 512:1024], j0 + 1)
                    es = esp.tile([128, 1024], BF16, tag="es")
                    nc.scalar.activation(
                        es[:], psc[:], mybir.ActivationFunctionType.Exp, scale=ESCL
                    )

                    def av(es=es, j0=j0):
                        av_mm(pav, es[:, 0:512], j0)
                        av_mm(pav, es[:, 512:1024], j0 + 1)

                    yield av
                for j in range(4 * q, 4 * q + 4):  # diagonal tiles
                    d = j - 4 * q
                    off = 128 * d
                    # scores/exp/attn@v only need columns that can be valid
                    psc = mmps.tile([128, 512], F32, tag="mm", name="pscd")
                    sc_mm(psc[:, off:512], j, off)
                    es = esp.tile([128, 1024], BF16, tag="es")
                    nc.scalar.activation(
                        es[:, off:512],
                        psc[:, off:512],
                        mybir.ActivationFunctionType.Exp,
                        scale=ESCL,
                    )
                    # keep tq - tk >= 0. Only the 128-col window [off, off+128)
                    # of the columns av reads can be invalid: local col f in
                    # that window is valid iff f - part >= 0.
                    nc.gpsimd.affine_select(
                        out=es[:, off : off + 128],
                        in_=es[:, off : off + 128],
                        compare_op=mybir.AluOpType.is_ge,
                        fill=0.0,
                        base=0,
                        pattern=[[1, 128]],
                        channel_multiplier=-1,
                    )

                    def av(es=es, j=j, off=off):
                        av_mm(pav, es[:, off:512], j, off)

                    yield av

            def normalize(h, q, pav):
                p, s = h // 2, h % 2
                r0, r1 = 64 * s, 64 * s + 64
                rcp = nrm.tile([1, 512], F32, tag="rcp")
                nc.vector.reciprocal(rcp[:], pav[64:65, :])
                rb = nrm.tile([64, 512], F32, tag="rb")
                nc.gpsimd.partition_broadcast(rb[:], rcp[0:1, :])
                nc.vector.tensor_mul(attnout[p][q][r0:r1, :], pav[0:64, :], rb[:])

            ysb_open = {}

            def outproj_group(q, ts, oc):
                t = q * 4 + ts
                py = mmps.tile([128, 512], F32, tag="mm")
                for p in range(2):
                    nc.tensor.matmul(
                        py[:],
                        attnout[p][q][:, ts * 128 : (ts + 1) * 128],
                        wout_t[p][:, oc * 512 : (oc + 1) * 512],
                        start=(p == 0),
                        stop=(p == 1),
                    )
                if oc == 0:
                    ysb = yop.tile([128, 1024], BF16, tag="y", name=f"y{q}_{ts}")
                    ysb_open[(q, ts)] = ysb
                else:
                    ysb = ysb_open.pop((q, ts))
                # split the PSUM->SBUF drain across DVE and Pool so neither
                # lane becomes the bottleneck
                if oc == 0:
                    nc.vector.tensor_copy(ysb[:, oc * 512 : (oc + 1) * 512], py[:])
                else:
                    nc.gpsimd.tensor_copy(ysb[:, oc * 512 : (oc + 1) * 512], py[:])
                if oc == 1:
                    nc.sync.dma_start(y[t * 128 : (t + 1) * 128, :], ysb[:])

            # Flat emission, load-levelled: projection chunks n>=2 interleave
            # with early attention chunks; out-projection groups are sprinkled
            # between units as PE filler for the ACT-paced late chunks. A
            # chunk's outproj groups become eligible only once its last unit's
            # normalize has been EMITTED (Tile derives dependencies from
            # program order).
            pending = None  # (av_thunk, normalize_thunk, after_thunks)
            pending_outproj = []  # eligible outproj group thunks

            def flush_pending():
                nonlocal pending
                if pending is not None:
                    pending[0]()
                    pending[1]()
                    pending_outproj.extend(pending[2])
                    pending = None

            def attn_chunk(q):
                nonlocal pending
                for p in range(2):
                    attnout[p][q] = aop.tile(
                        [128, 512], BF16, tag=f"ao{p}_{q}", name=f"ao{p}_{q}"
                    )
                for h in range(HPC):
                    pav = avps.tile([65, 512], F32, tag="av")
                    prev_av = None
                    # Off-diagonal pair blocks are ACT-paced (exp 1038ns vs
                    # PE 853ns): drop one outproj group between every other
                    # pair block as PE filler.
                    for bi, av in enumerate(emit_unit_blocks(h, q, pav)):
                        if prev_av is not None:
                            prev_av()
                            if bi < 2 * q and bi % 2 == 1 and pending_outproj:
                                pending_outproj.pop(0)()
                        elif pending is not None:
                            flush_pending()
                        prev_av = av
                    after = (
                        [
                            (lambda q=q, ts=ts, oc=oc: outproj_group(q, ts, oc))
                            for ts in range(4)
                            for oc in range(2)
                        ]
                        if h == HPC - 1
                        else []
                    )
                    pending = (
                        prev_av,
                        lambda h=h, q=q, pav=pav: normalize(h, q, pav),
                        after,
                    )
                    for _ in range(2):
                        if pending_outproj:
                            pending_outproj.pop(0)()

            proj_chunk(0)
            proj_chunk(1)
            emit_wout_load()
            attn_chunk(0)
            proj_chunk(2)
            attn_chunk(1)
            proj_chunk(3)
            attn_chunk(2)
            attn_chunk(3)
            flush_pending()
            for th in pending_outproj:
                th()

    nc.finalize()
    return nc


def _prep_core_inputs(x, w_qkv, w_out, core):
    import ml_dtypes

    BF = ml_dtypes.bfloat16
    F8 = ml_dtypes.float8_e4m3
    b, hg = core // HG, core % HG
    xT = np.ascontiguousarray(x[b].T)  # [C, T]
    wq = w_qkv[0:C] * np.float32(SQ / np.sqrt(D))
    wk = w_qkv[C : 2 * C] * np.float32(SK)
    wv = w_qkv[2 * C : 3 * C]
    h0 = HPC * hg
    rows = []
    for p in range(2):
        rows.append(wq[64 * (h0 + 2 * p) : 64 * (h0 + 2 * p + 2)])
    for p in range(2):
        rows.append(wk[64 * (h0 + 2 * p) : 64 * (h0 + 2 * p + 2)])
    # [C, 512] column-stacked lhsT, then fp8 DoubleRow layout:
    # [kk(4) x i(2) x p(128), m(4) x c(128)] -> [128, kk, 4(m), 2(i), 128]
    wqk_lhsT = np.concatenate(rows, axis=0).T  # [C, 512]
    wqk8 = (
        np.ascontiguousarray(
            wqk_lhsT.reshape(KK, 2, 128, 4, 128).transpose(2, 0, 3, 1, 4)
        )
        .reshape(128, KK * 1024)
        .astype(F8)
    )
    # x8: [C, T] -> [128, kk*2+i, T] with c = kk*256 + i*128 + p
    x8 = (
        np.ascontiguousarray(xT.reshape(KK, 2, 128, T).transpose(2, 0, 1, 3))
        .reshape(128, 2 * KK, T)
        .astype(F8)
    )
    wv_rhsT = np.ascontiguousarray(wv[64 * h0 : 64 * (h0 + HPC)].T)
    wout_pairs = np.ascontiguousarray(
        w_out[:, 64 * h0 : 64 * (h0 + HPC)].T
    ).reshape(2, 128, C)
    # xT and wv are sent p-major shuffled ([128, CK, ...]) so the kernel can
    # load several contraction k-tiles with one contiguous DMA.
    xTs = np.ascontiguousarray(xT.reshape(CK, 128, T).transpose(1, 0, 2))
    wvs = np.ascontiguousarray(wv_rhsT.reshape(CK, 128, 256).transpose(1, 0, 2))
    return {
        "ones": np.ones((128, 4), dtype=BF),
        "xT": xTs.astype(BF),
        "x8": x8,
        "wqk8": wqk8,
        "wv": wvs.astype(BF),
        "wout": wout_pairs.astype(BF),
    }


def kernel(x, w_qkv, w_out):
    from concourse.bass_utils import run_bass_kernel_spmd

    global _NC
    x = np.asarray(x, dtype=np.float32)
    w_qkv = np.asarray(w_qkv, dtype=np.float32)
    w_out = np.asarray(w_out, dtype=np.float32)

    in_maps = [_prep_core_inputs(x, w_qkv, w_out, c) for c in range(N_CORES)]
    if _NC is None:
        _NC = _build_nc()
    res = run_bass_kernel_spmd(_NC, in_maps, core_ids=list(range(N_CORES)))
    out = np.zeros((B, T, C), dtype=np.float32)
    for c in range(N_CORES):
        out[c // HG] += np.asarray(res.results[c]["y"]).astype(np.float32)
    return out


# revision 14
# speedup vs baseline: 1.0352x; 1.0352x over previous
"""Causal self-attention (B=2, T=2048, C=1024, H=16, D=64) on 8 Trainium2
NeuronCores.

Sharding: core = (batch, head-group): b = core // 4, hg = core % 4; each core
computes 4 heads of one batch plus its partial out-projection (256 of 1024
contraction channels). Host sums the 4 partial y's per batch.

Per-core pipeline:
  1. qT/kT = (w_qk.T).T @ xT via fp8e4m3 DoubleRow matmuls (K=256 per
     instruction, 0.5 cycles/row): host pre-scales wq by 256/sqrt(D) and wk
     by 64 so the fp8 weights sit in-range; the 1/16384 descale is folded
     into the exp's scale operand. v = xT.T @ wv.T in bf16 (the value path
     cannot afford fp8 error).
  2. scores^T[tk, tq] = kT.T @ qT per head in bf16, block [128, 512], causal
     block-skip; diagonal blocks trimmed to the valid column range (bf16 has
     no >=256 moving-size requirement). exp on ACT (PSUM->SBUF, bf16 out,
     scale=1/16384); causal mask via gpsimd affine_select restricted to the
     single 128-col window per diagonal tile that av actually reads.
  3. outT[65, tq] = [v|1].T @ expS^T accumulated over tk tiles (bf16); row 64
     is the softmax denominator. Normalize: reciprocal + partition_broadcast
     + tensor_mul into pair-stacked attnout^T tiles [128, 512] bf16.
  4. y[t, o] += attnout^T.T @ w_out^T per head-pair (K=128, bf16), PSUM
     DMA'd directly to DRAM (no SBUF staging).
"""

import numpy as np

B, T, C = 2, 2048, 1024
H, D = 16, 64
N_CORES = 8
HG = 4  # head-groups (cores per batch)
HPC = 4  # heads per core
NCH = T // 512  # 4 tq chunks of 512
KT = T // 128  # 16 tk tiles of 128
CK = C // 128  # 8 contraction k-tiles for the bf16 projections
KK = C // 256  # 4 fp8 DoubleRow k-tiles (K=256 each) for the qk projection
SQ = 256.0  # wq prescale (includes 1/sqrt(D) fold)
SK = 64.0  # wk prescale
ESCL = 1.0 / (SQ * SK)  # exp descale

_NC = None


def _build_nc():
    import concourse.mybir as mybir
    import concourse.tile as tile
    from concourse import bacc

    F32 = mybir.dt.float32
    BF16 = mybir.dt.bfloat16
    F8 = mybir.dt.float8e4
    DR = mybir.MatmulPerfMode.DoubleRow

    nc = bacc.Bacc(None, target_bir_lowering=False)
    xT = nc.dram_tensor("xT", [128, CK, T], BF16, kind="ExternalInput")
    x8 = nc.dram_tensor("x8", [128, 2 * KK, T], F8, kind="ExternalInput")
    wqk8 = nc.dram_tensor("wqk8", [128, KK * 1024], F8, kind="ExternalInput")
    wv = nc.dram_tensor("wv", [128, CK, 256], BF16, kind="ExternalInput")
    wout = nc.dram_tensor("wout", [2, 128, C], BF16, kind="ExternalInput")
    ones = nc.dram_tensor("ones", [128, 4], BF16, kind="ExternalInput")
    y = nc.dram_tensor("y", [T, C], BF16, kind="ExternalOutput")

    with tile.TileContext(nc) as tc:
        with (
            tc.tile_pool(name="const", bufs=1) as const,
            tc.tile_pool(name="xin", bufs=3) as xin,
            tc.tile_pool(name="x8in", bufs=3) as x8in,
            tc.tile_pool(name="qk", bufs=1) as qkp,
            tc.tile_pool(name="vt", bufs=1) as vtp,
            tc.tile_pool(name="es", bufs=8) as esp,
            tc.tile_pool(name="ao", bufs=1) as aop,
            tc.tile_pool(name="nrm", bufs=3) as nrm,
            tc.tile_pool(name="yo", bufs=4) as yop,
            tc.tile_pool(name="mm", bufs=2, space="PSUM") as mmps,
            tc.tile_pool(name="sc", bufs=2, space="PSUM") as scps,
            tc.tile_pool(name="av", bufs=2, space="PSUM") as avps,
        ):
            # ---- weights + first x chunk. DMA queue order is the startup
            # critical path: qk weights + first x8 halves first (DoubleRow
            # proj can begin ~2us in), then wv + xt for the v projection;
            # wout is deferred to after proj chunk 1 (first needed ~30us in).
            wqk8_sb = const.tile([128, KK * 1024], F8, tag="wqk8")
            nc.sync.dma_start(wqk8_sb[:, 0 : 2 * 1024], wqk8[:, 0 : 2 * 1024])
            x8_first = x8in.tile([128, KK * 1024], F8, tag="x8", name="x8_0")
            nc.sync.dma_start(
                x8_first[:, 0:2048].rearrange("p (g t) -> p g t", g=4),
                x8[:, 0:4, 0:512],
            )
            nc.sync.dma_start(wqk8_sb[:, 2 * 1024 :], wqk8[:, 2 * 1024 :])
            nc.sync.dma_start(
                x8_first[:, 2048:].rearrange("p (g t) -> p g t", g=4),
                x8[:, 4:8, 0:512],
            )
            wv_all = const.tile([128, CK * 256], BF16, tag="wvall")
            nc.sync.dma_start(
                wv_all[:].rearrange("p (g c) -> p g c", g=CK), wv[:, :, :]
            )
            wv_t = [wv_all[:, k * 256 : (k + 1) * 256] for k in range(CK)]
            xt_first = []
            for k in range(CK):
                t_ = xin.tile([128, 512], BF16, tag=f"xt{k}", name=f"xt0_{k}", bufs=1)
                nc.sync.dma_start(t_[:], xT[:, k, 0:512])
                xt_first.append(t_)
            ones_sb = const.tile([128, 4], BF16, tag="ones")
            nc.sync.dma_start(ones_sb[:], ones[:])

            # wqk8 lhsT access for (kk, m): [128, 2, 128]
            def wqk8_ap(kk, m):
                return wqk8_sb[
                    :, kk * 1024 + m * 256 : kk * 1024 + (m + 1) * 256
                ].rearrange("p (i c) -> p i c", i=2)

            # ---- v tiles with ones columns ----
            v_t = []
            for t in range(KT):
                t_ = vtp.tile([128, 4 * 65], BF16, tag=f"v{t}")
                nc.vector.tensor_copy(
                    t_[:].rearrange("p (h e) -> p h e", h=4)[:, :, 64:65],
                    ones_sb[:].rearrange("p (h e) -> p h e", e=1),
                )
                v_t.append(t_)

            # ---- projections, streamed by tq/n chunk ----
            # qkT[m] tiles per (m, n): m 0..1 = q head-pairs, 2..3 = k pairs
            qkT = [[None] * NCH for _ in range(4)]

            def proj_chunk(n):
                if n == 0:
                    x8_n = x8_first
                    xt_n = xt_first
                else:
                    x8_n = x8in.tile(
                        [128, KK * 1024], F8, tag="x8", name=f"x8_{n}", bufs=3
                    )
                    nc.sync.dma_start(
                        x8_n[:].rearrange("p (g t) -> p g t", g=2 * KK),
                        x8[:, :, n * 512 : (n + 1) * 512],
                    )
                    xt_n = []
                    for half in range(2):
                        big = xin.tile(
                            [128, 4 * 512], BF16, tag=f"xth{half}",
                            name=f"xth{half}_{n}", bufs=2,
                        )
                        nc.sync.dma_start(
                            big[:].rearrange("p (g t) -> p g t", g=4),
                            xT[:, half * 4 : half * 4 + 4, n * 512 : (n + 1) * 512],
                        )
                        xt_n += [big[:, k * 512 : (k + 1) * 512] for k in range(4)]
                for m in range(4):
                    ps = mmps.tile([128, 512], F32, tag="mm")
                    for kk in range(KK):
                        nc.tensor.matmul(
                            ps[:],
                            wqk8_ap(kk, m),
                            x8_n[
                                :, kk * 1024 : (kk + 1) * 1024
                            ].rearrange("p (i t) -> p i t", i=2),
                            start=(kk == 0),
                            stop=(kk == KK - 1),
                            perf_mode=DR,
                        )
                    sb = qkp.tile([128, 512], BF16, tag=f"qk{m}_{n}", name=f"qk{m}_{n}")
                    nc.vector.tensor_copy(sb[:], ps[:])
                    qkT[m][n] = sb
                for ts in range(4):  # t-subtiles of this chunk
                    t = n * 4 + ts
                    ps = mmps.tile([128, 256], F32, tag="mm")
                    for k in range(CK):
                        nc.tensor.matmul(
                            ps[:],
                            xt_n[k][:, ts * 128 : (ts + 1) * 128],
                            wv_t[k][:],
                            start=(k == 0),
                            stop=(k == CK - 1),
                        )
                    nc.vector.tensor_copy(
                        v_t[t][:].rearrange("p (h e) -> p h e", h=4)[:, :, 0:64],
                        ps[:].rearrange("p (h d) -> p h d", h=4),
                    )

            # ---- attention + out-projection, per tq chunk ----
            # Blocks are software-pipelined: scores+exp run one block ahead of
            # the attn@v accumulation so PE never stalls on ACT. Off-diagonal
            # tk tiles are computed in pairs sharing one [128,1024] exp; the 4
            # diagonal tiles get exps/scores trimmed to the column range that
            # can be valid, and a 128-col affine_select causal window.
            attnout = [[None] * NCH for _ in range(2)]
            wout_t = []

            def emit_wout_load():
                for p in range(2):
                    t_ = const.tile([128, C], BF16, tag=f"wout{p}")
                    nc.sync.dma_start(t_[:], wout[p])
                    wout_t.append(t_)

            def emit_unit_blocks(h, q, pav):
                """Yields av-emit thunks, one per block, after emitting that
                block's scores+exp instructions."""
                p, s = h // 2, h % 2
                r0, r1 = 64 * s, 64 * s + 64
                vslice = slice(65 * h, 65 * h + 65)
                last_j = 4 * q + 3

                def sc_mm(out_ap, j, trim=0):
                    nc.tensor.matmul(
                        out_ap,
                        qkT[2 + p][j // 4][r0:r1, (j % 4) * 128 : (j % 4 + 1) * 128],
                        qkT[p][q][r0:r1, trim:512],
                        start=True,
                        stop=True,
                    )

                def av_mm(pav, es_ap, j, trim=0):
                    nc.tensor.matmul(
                        pav[:, trim:512],
                        v_t[j][:, vslice],
                        es_ap,
                        start=(j == 0),
                        stop=(j == last_j),
                    )

                for j0 in range(0, 4 * q, 2):  # off-diagonal pairs
                    psc = scps.tile([128, 1024], F32, tag="sc")
                    sc_mm(psc[:, 0:512], j0)
                    sc_mm(psc[:, 512:1024], j0 + 1)
                    es = esp.tile([128, 1024], BF16, tag="es")
                    nc.scalar.activation(
                        es[:], psc[:], mybir.ActivationFunctionType.Exp, scale=ESCL
                    )

                    def av(es=es, j0=j0):
                        av_mm(pav, es[:, 0:512], j0)
                        av_mm(pav, es[:, 512:1024], j0 + 1)

                    yield av
                for j in range(4 * q, 4 * q + 4):  # diagonal tiles
                    d = j - 4 * q
                    off = 128 * d
                    # scores/exp/attn@v only need columns that can be valid
                    psc = mmps.tile([128, 512], F32, tag="mm", name="pscd")
                    sc_mm(psc[:, off:512], j, off)
                    es = esp.tile([128, 1024], BF16, tag="es")
                    nc.scalar.activation(
                        es[:, off:512],
                        psc[:, off:512],
                        mybir.ActivationFunctionType.Exp,
                        scale=ESCL,
                    )
                    # keep tq - tk >= 0. Only the 128-col window [off, off+128)
                    # of the columns av reads can be invalid: local col f in
                    # that window is valid iff f - part >= 0.
                    nc.gpsimd.affine_select(
                        out=es[:, off : off + 128],
                        in_=es[:, off : off + 128],
                        compare_op=mybir.AluOpType.is_ge,
                        fill=0.0,
                        base=0,
                        pattern=[[1, 128]],
                        channel_multiplier=-1,
                    )

                    def av(es=es, j=j, off=off):
                        av_mm(pav, es[:, off:512], j, off)

                    yield av

            def normalize(h, q, pav):
                p, s = h // 2, h % 2
                r0, r1 = 64 * s, 64 * s + 64
                rcp = nrm.tile([1, 512], F32, tag="rcp")
                nc.vector.reciprocal(rcp[:], pav[64:65, :])
                rb = nrm.tile([64, 512], F32, tag="rb")
                nc.gpsimd.partition_broadcast(rb[:], rcp[0:1, :])
                nc.vector.tensor_mul(attnout[p][q][r0:r1, :], pav[0:64, :], rb[:])

            ysb_open = {}

            def outproj_group(q, ts, oc):
                t = q * 4 + ts
                py = mmps.tile([128, 512], F32, tag="mm")
                for p in range(2):
                    nc.tensor.matmul(
                        py[:],
                        attnout[p][q][:, ts * 128 : (ts + 1) * 128],
                        wout_t[p][:, oc * 512 : (oc + 1) * 512],
                        start=(p == 0),
                        stop=(p == 1),
                    )
                if oc == 0:
                    ysb = yop.tile([128, 1024], BF16, tag="y", name=f"y{q}_{ts}")
                    ysb_open[(q, ts)] = ysb
                else:
                    ysb = ysb_open.pop((q, ts))
                # split the PSUM->SBUF drain across DVE and Pool so neither
                # lane becomes the bottleneck
                if oc == 0:
                    nc.vector.tensor_copy(ysb[:, oc * 512 : (oc + 1) * 512], py[:])
                else:
                    nc.gpsimd.tensor_copy(ysb[:, oc * 512 : (oc + 1) * 512], py[:])
                if oc == 1:
                    nc.sync.dma_start(y[t * 128 : (t + 1) * 128, :], ysb[:])

            # Flat emission, load-levelled: projection chunks n>=2 interleave
            # with early attention chunks; out-projection groups are sprinkled
            # between units as PE filler for the ACT-paced late chunks. A
            # chunk's outproj groups become eligible only once its last unit's
            # normalize has been EMITTED (Tile derives dependencies from
            # program order).
            pending = None  # (av_thunk, normalize_thunk, after_thunks)
            pending_outproj = []  # eligible outproj group thunks

            def flush_pending():
                nonlocal pending
                if pending is not None:
                    pending[0]()
                    pending[1]()
                    pending_outproj.extend(pending[2])
                    pending = None

            def attn_chunk(q):
                nonlocal pending
                for p in range(2):
                    attnout[p][q] = aop.tile(
                        [128, 512], BF16, tag=f"ao{p}_{q}", name=f"ao{p}_{q}"
                    )
                for h in range(HPC):
                    pav = avps.tile([65, 512], F32, tag="av")
                    prev_av = None
                    for bi, av in enumerate(emit_unit_blocks(h, q, pav)):
                        if prev_av is not None:
                            prev_av()
                        elif pending is not None:
                            flush_pending()
                        prev_av = av
                    after = (
                        [
                            (lambda q=q, ts=ts, oc=oc: outproj_group(q, ts, oc))
                            for ts in range(4)
                            for oc in range(2)
                        ]
                        if h == HPC - 1
                        else []
                    )
                    pending = (
                        prev_av,
                        lambda h=h, q=q, pav=pav: normalize(h, q, pav),
                        after,
                    )
                    for _ in range(2):
                        if pending_outproj:
                            pending_outproj.pop(0)()

            proj_chunk(0)
            proj_chunk(1)
            emit_wout_load()
            attn_chunk(0)
            proj_chunk(2)
            attn_chunk(1)
            proj_chunk(3)
            attn_chunk(2)
            attn_chunk(3)
            flush_pending()
            for th in pending_outproj:
                th()

    nc.finalize()
    return nc


def _prep_core_inputs(x, w_qkv, w_out, core):
    import ml_dtypes

    BF = ml_dtypes.bfloat16
    F8 = ml_dtypes.float8_e4m3
    b, hg = core // HG, core % HG
    xT = np.ascontiguousarray(x[b].T)  # [C, T]
    wq = w_qkv[0:C] * np.float32(SQ / np.sqrt(D))
    wk = w_qkv[C : 2 * C] * np.float32(SK)
    wv = w_qkv[2 * C : 3 * C]
    h0 = HPC * hg
    rows = []
    for p in range(2):
        rows.append(wq[64 * (h0 + 2 * p) : 64 * (h0 + 2 * p + 2)])
    for p in range(2):
        rows.append(wk[64 * (h0 + 2 * p) : 64 * (h0 + 2 * p + 2)])
    # [C, 512] column-stacked lhsT, then fp8 DoubleRow layout:
    # [kk(4) x i(2) x p(128), m(4) x c(128)] -> [128, kk, 4(m), 2(i), 128]
    wqk_lhsT = np.concatenate(rows, axis=0).T  # [C, 512]
    wqk8 = (
        np.ascontiguousarray(
            wqk_lhsT.reshape(KK, 2, 128, 4, 128).transpose(2, 0, 3, 1, 4)
        )
        .reshape(128, KK * 1024)
        .astype(F8)
    )
    # x8: [C, T] -> [128, kk*2+i, T] with c = kk*256 + i*128 + p
    x8 = (
        np.ascontiguousarray(xT.reshape(KK, 2, 128, T).transpose(2, 0, 1, 3))
        .reshape(128, 2 * KK, T)
        .astype(F8)
    )
    wv_rhsT = np.ascontiguousarray(wv[64 * h0 : 64 * (h0 + HPC)].T)
    wout_pairs = np.ascontiguousarray(
        w_out[:, 64 * h0 : 64 * (h0 + HPC)].T
    ).reshape(2, 128, C)
    # xT and wv are sent p-major shuffled ([128, CK, ...]) so the kernel can
    # load several contraction k-tiles with one contiguous DMA.
    xTs = np.ascontiguousarray(xT.reshape(CK, 128, T).transpose(1, 0, 2))
    wvs = np.ascontiguousarray(wv_rhsT.reshape(CK, 128, 256).transpose(1, 0, 2))
    return {
        "ones": np.ones((128, 4), dtype=BF),
        "xT": xTs.astype(BF),
        "x8": x8,
        "wqk8": wqk8,
        "wv": wvs.astype(BF),
        "wout": wout_pairs.astype(BF),
    }


def kernel(x, w_qkv, w_out):
    from concourse.bass_utils import run_bass_kernel_spmd

    global _NC
    x = np.asarray(x, dtype=np.float32)
    w_qkv = np.asarray(w_qkv, dtype=np.float32)
    w_out = np.asarray(w_out, dtype=np.float32)

    in_maps = [_prep_core_inputs(x, w_qkv, w_out, c) for c in range(N_CORES)]
    if _NC is None:
        _NC = _build_nc()
    res = run_bass_kernel_spmd(_NC, in_maps, core_ids=list(range(N_CORES)))
    out = np.zeros((B, T, C), dtype=np.float32)
    for c in range(N_CORES):
        out[c // HG] += np.asarray(res.results[c]["y"]).astype(np.float32)
    return out


# revision 20
# speedup vs baseline: 1.0498x; 1.0141x over previous
"""Causal self-attention (B=2, T=2048, C=1024, H=16, D=64) on 8 Trainium2
NeuronCores.

Sharding: core = (batch, head-group): b = core // 4, hg = core % 4; each core
computes 4 heads of one batch plus its partial out-projection (256 of 1024
contraction channels). Host sums the 4 partial y's per batch.

Per-core pipeline:
  1. qT/kT = (w_qk.T).T @ xT via fp8e4m3 DoubleRow matmuls (K=256 per
     instruction, 0.5 cycles/row): host pre-scales wq by 256/sqrt(D) and wk
     by 64 so the fp8 weights sit in-range; the 1/16384 descale is folded
     into the exp's scale operand. v = xT.T @ wv.T in bf16 (the value path
     cannot afford fp8 error).
  2. scores^T[tk, tq] = kT.T @ qT per head in bf16, block [128, 512], causal
     block-skip; diagonal blocks trimmed to the valid column range (bf16 has
     no >=256 moving-size requirement). exp on ACT (PSUM->SBUF, bf16 out,
     scale=1/16384); causal mask via gpsimd affine_select restricted to the
     single 128-col window per diagonal tile that av actually reads.
  3. outT[65, tq] = [v|1].T @ expS^T accumulated over tk tiles (bf16); row 64
     is the softmax denominator. Normalize: reciprocal + partition_broadcast
     + tensor_mul into pair-stacked attnout^T tiles [128, 512] bf16.
  4. y[t, o] += attnout^T.T @ w_out^T per head-pair (K=128, bf16), PSUM
     DMA'd directly to DRAM (no SBUF staging).
"""

import numpy as np

B, T, C = 2, 2048, 1024
H, D = 16, 64
N_CORES = 8
HG = 4  # head-groups (cores per batch)
HPC = 4  # heads per core
NCH = T // 512  # 4 tq chunks of 512
KT = T // 128  # 16 tk tiles of 128
CK = C // 128  # 8 contraction k-tiles for the bf16 projections
KK = C // 256  # 4 fp8 DoubleRow k-tiles (K=256 each) for the qk projection
SQ = 256.0  # wq prescale (includes 1/sqrt(D) fold)
SK = 64.0  # wk prescale
ESCL = 1.0 / (SQ * SK)  # exp descale

_NC = None


def _build_nc():
    import concourse.mybir as mybir
    import concourse.tile as tile
    from concourse import bacc

    F32 = mybir.dt.float32
    BF16 = mybir.dt.bfloat16
    F8 = mybir.dt.float8e4
    DR = mybir.MatmulPerfMode.DoubleRow

    nc = bacc.Bacc(None, target_bir_lowering=False)
    xT = nc.dram_tensor("xT", [128, CK, T], BF16, kind="ExternalInput")
    x8 = nc.dram_tensor("x8", [128, 2 * KK, T], F8, kind="ExternalInput")
    wqk8 = nc.dram_tensor("wqk8", [128, KK * 1024], F8, kind="ExternalInput")
    wv = nc.dram_tensor("wv", [128, CK, 256], BF16, kind="ExternalInput")
    wout = nc.dram_tensor("wout", [2, 128, C], BF16, kind="ExternalInput")
    ones = nc.dram_tensor("ones", [128, 4], BF16, kind="ExternalInput")
    y = nc.dram_tensor("y", [T, C], BF16, kind="ExternalOutput")

    with tile.TileContext(nc) as tc:
        with (
            tc.tile_pool(name="const", bufs=1) as const,
            tc.tile_pool(name="xin", bufs=3) as xin,
            tc.tile_pool(name="x8in", bufs=3) as x8in,
            tc.tile_pool(name="qk", bufs=1) as qkp,
            tc.tile_pool(name="vt", bufs=1) as vtp,
            tc.tile_pool(name="es", bufs=8) as esp,
            tc.tile_pool(name="ao", bufs=1) as aop,
            tc.tile_pool(name="nrm", bufs=3) as nrm,
            tc.tile_pool(name="yo", bufs=4) as yop,
            tc.tile_pool(name="mm", bufs=2, space="PSUM") as mmps,
            tc.tile_pool(name="sc", bufs=2, space="PSUM") as scps,
            tc.tile_pool(name="av", bufs=2, space="PSUM") as avps,
        ):
            # ---- weights + first x chunk. DMA queue order is the startup
            # critical path: qk weights + first x8 halves first (DoubleRow
            # proj can begin ~2us in), then wv + xt for the v projection;
            # wout is deferred to after proj chunk 1 (first needed ~30us in).
            wqk8_sb = const.tile([128, KK * 1024], F8, tag="wqk8")
            nc.sync.dma_start(wqk8_sb[:, 0:1024], wqk8[:, 0:1024])
            x8_first = x8in.tile([128, KK * 1024], F8, tag="x8", name="x8_0")
            nc.sync.dma_start(
                x8_first[:, 0:1024].rearrange("p (g t) -> p g t", g=2),
                x8[:, 0:2, 0:512],
            )
            nc.sync.dma_start(wqk8_sb[:, 1024:], wqk8[:, 1024:])
            nc.sync.dma_start(
                x8_first[:, 1024:].rearrange("p (g t) -> p g t", g=6),
                x8[:, 2:8, 0:512],
            )
            wv_all = const.tile([128, CK * 256], BF16, tag="wvall")
            nc.sync.dma_start(
                wv_all[:].rearrange("p (g c) -> p g c", g=CK), wv[:, :, :]
            )
            wv_t = [wv_all[:, k * 256 : (k + 1) * 256] for k in range(CK)]
            xt_first = []
            for half in range(2):
                big = xin.tile(
                    [128, 4 * 512], BF16, tag=f"xth{half}", name=f"xth{half}_0",
                    bufs=2,
                )
                nc.sync.dma_start(
                    big[:].rearrange("p (g t) -> p g t", g=4),
                    xT[:, half * 4 : half * 4 + 4, 0:512],
                )
                xt_first += [big[:, k * 512 : (k + 1) * 512] for k in range(4)]
            ones_sb = const.tile([128, 4], BF16, tag="ones")
            nc.sync.dma_start(ones_sb[:], ones[:])

            # wqk8 lhsT access for (kk, m): [128, 2, 128]
            def wqk8_ap(kk, m):
                return wqk8_sb[
                    :, kk * 1024 + m * 256 : kk * 1024 + (m + 1) * 256
                ].rearrange("p (i c) -> p i c", i=2)

            # ---- v tiles with ones columns ----
            v_t = []
            for t in range(KT):
                t_ = vtp.tile([128, 4 * 65], BF16, tag=f"v{t}")
                nc.vector.tensor_copy(
                    t_[:].rearrange("p (h e) -> p h e", h=4)[:, :, 64:65],
                    ones_sb[:].rearrange("p (h e) -> p h e", e=1),
                )
                v_t.append(t_)

            # ---- projections, streamed by tq/n chunk ----
            # qkT[m] tiles per (m, n): m 0..1 = q head-pairs, 2..3 = k pairs
            qkT = [[None] * NCH for _ in range(4)]

            def proj_chunk(n):
                if n == 0:
                    x8_n = x8_first
                    xt_n = xt_first
                else:
                    x8_n = x8in.tile(
                        [128, KK * 1024], F8, tag="x8", name=f"x8_{n}", bufs=3
                    )
                    nc.sync.dma_start(
                        x8_n[:].rearrange("p (g t) -> p g t", g=2 * KK),
                        x8[:, :, n * 512 : (n + 1) * 512],
                    )
                    xt_n = []
                    for half in range(2):
                        big = xin.tile(
                            [128, 4 * 512], BF16, tag=f"xth{half}",
                            name=f"xth{half}_{n}", bufs=2,
                        )
                        nc.sync.dma_start(
                            big[:].rearrange("p (g t) -> p g t", g=4),
                            xT[:, half * 4 : half * 4 + 4, n * 512 : (n + 1) * 512],
                        )
                        xt_n += [big[:, k * 512 : (k + 1) * 512] for k in range(4)]
                for m in range(4):
                    ps = mmps.tile([128, 512], F32, tag="mm")
                    for kk in range(KK):
                        nc.tensor.matmul(
                            ps[:],
                            wqk8_ap(kk, m),
                            x8_n[
                                :, kk * 1024 : (kk + 1) * 1024
                            ].rearrange("p (i t) -> p i t", i=2),
                            start=(kk == 0),
                            stop=(kk == KK - 1),
                            perf_mode=DR,
                        )
                    sb = qkp.tile([128, 512], BF16, tag=f"qk{m}_{n}", name=f"qk{m}_{n}")
                    nc.vector.tensor_copy(sb[:], ps[:])
                    qkT[m][n] = sb
                for ts in range(4):  # t-subtiles of this chunk
                    t = n * 4 + ts
                    ps = mmps.tile([128, 256], F32, tag="mm")
                    for k in range(CK):
                        nc.tensor.matmul(
                            ps[:],
                            xt_n[k][:, ts * 128 : (ts + 1) * 128],
                            wv_t[k][:],
                            start=(k == 0),
                            stop=(k == CK - 1),
                        )
                    nc.vector.tensor_copy(
                        v_t[t][:].rearrange("p (h e) -> p h e", h=4)[:, :, 0:64],
                        ps[:].rearrange("p (h d) -> p h d", h=4),
                    )

            # ---- attention + out-projection, per tq chunk ----
            # Blocks are software-pipelined: scores+exp run one block ahead of
            # the attn@v accumulation so PE never stalls on ACT. Off-diagonal
            # tk tiles are computed in pairs sharing one [128,1024] exp; the 4
            # diagonal tiles get exps/scores trimmed to the column range that
            # can be valid, and a 128-col affine_select causal window.
            attnout = [[None] * NCH for _ in range(2)]
            wout_t = []

            def emit_wout_load():
                for p in range(2):
                    t_ = const.tile([128, C], BF16, tag=f"wout{p}")
                    nc.sync.dma_start(t_[:], wout[p])
                    wout_t.append(t_)

            def emit_unit_blocks(h, q, pav):
                """Yields av-emit thunks, one per block, after emitting that
                block's scores+exp instructions."""
                p, s = h // 2, h % 2
                r0, r1 = 64 * s, 64 * s + 64
                vslice = slice(65 * h, 65 * h + 65)
                last_j = 4 * q + 3

                def sc_mm(out_ap, j, trim=0):
                    nc.tensor.matmul(
                        out_ap,
                        qkT[2 + p][j // 4][r0:r1, (j % 4) * 128 : (j % 4 + 1) * 128],
                        qkT[p][q][r0:r1, trim:512],
                        start=True,
                        stop=True,
                    )

                def av_mm(pav, es_ap, j, trim=0):
                    nc.tensor.matmul(
                        pav[:, trim:512],
                        v_t[j][:, vslice],
                        es_ap,
                        start=(j == 0),
                        stop=(j == last_j),
                    )

                for j0 in range(0, 4 * q, 2):  # off-diagonal pairs
                    psc = scps.tile([128, 1024], F32, tag="sc")
                    sc_mm(psc[:, 0:512], j0)
                    sc_mm(psc[:, 512:1024], j0 + 1)
                    es = esp.tile([128, 1024], BF16, tag="es")
                    nc.scalar.activation(
                        es[:], psc[:], mybir.ActivationFunctionType.Exp, scale=ESCL
                    )

                    def av(es=es, j0=j0):
                        av_mm(pav, es[:, 0:512], j0)
                        av_mm(pav, es[:, 512:1024], j0 + 1)

                    yield av
                for j in range(4 * q, 4 * q + 4):  # diagonal tiles
                    d = j - 4 * q
                    off = 128 * d
                    # scores/exp/attn@v only need columns that can be valid
                    psc = mmps.tile([128, 512], F32, tag="mm", name="pscd")
                    sc_mm(psc[:, off:512], j, off)
                    es = esp.tile([128, 1024], BF16, tag="es")
                    nc.scalar.activation(
                        es[:, off:512],
                        psc[:, off:512],
                        mybir.ActivationFunctionType.Exp,
                        scale=ESCL,
                    )
                    # keep tq - tk >= 0. Only the 128-col window [off, off+128)
                    # of the columns av reads can be invalid: local col f in
                    # that window is valid iff f - part >= 0.
                    nc.gpsimd.affine_select(
                        out=es[:, off : off + 128],
                        in_=es[:, off : off + 128],
                        compare_op=mybir.AluOpType.is_ge,
                        fill=0.0,
                        base=0,
                        pattern=[[1, 128]],
                        channel_multiplier=-1,
                    )

                    def av(es=es, j=j, off=off):
                        av_mm(pav, es[:, off:512], j, off)

                    yield av

            def normalize(h, q, pav):
                p, s = h // 2, h % 2
                r0, r1 = 64 * s, 64 * s + 64
                rcp = nrm.tile([1, 512], F32, tag="rcp")
                nc.vector.reciprocal(rcp[:], pav[64:65, :])
                rb = nrm.tile([64, 512], F32, tag="rb")
                nc.gpsimd.partition_broadcast(rb[:], rcp[0:1, :])
                nc.vector.tensor_mul(attnout[p][q][r0:r1, :], pav[0:64, :], rb[:])

            ysb_open = {}

            def outproj_group(q, ts, oc, use_sc=False):
                t = q * 4 + ts
                if use_sc:
                    py_t = scps.tile([128, 1024], F32, tag="sc", name=f"yt{t}_{oc}")
                    py = py_t[:, 0:512]
                else:
                    py = mmps.tile([128, 512], F32, tag="mm")
                for p in range(2):
                    nc.tensor.matmul(
                        py[:],
                        attnout[p][q][:, ts * 128 : (ts + 1) * 128],
                        wout_t[p][:, oc * 512 : (oc + 1) * 512],
                        start=(p == 0),
                        stop=(p == 1),
                    )
                if oc == 0:
                    ysb = yop.tile([128, 1024], BF16, tag="y", name=f"y{q}_{ts}")
                    ysb_open[(q, ts)] = ysb
                else:
                    ysb = ysb_open.pop((q, ts))
                # split the PSUM->SBUF drain across DVE and Pool so neither
                # lane becomes the bottleneck
                if oc == 0:
                    nc.vector.tensor_copy(ysb[:, oc * 512 : (oc + 1) * 512], py[:])
                else:
                    nc.gpsimd.tensor_copy(ysb[:, oc * 512 : (oc + 1) * 512], py[:])
                if oc == 1:
                    nc.sync.dma_start(y[t * 128 : (t + 1) * 128, :], ysb[:])

            # Flat emission, load-levelled: projection chunks n>=2 interleave
            # with early attention chunks; out-projection groups are sprinkled
            # between units as PE filler for the ACT-paced late chunks. A
            # chunk's outproj groups become eligible only once its last unit's
            # normalize has been EMITTED (Tile derives dependencies from
            # program order).
            pending = None  # (av_thunk, normalize_thunk, after_thunks)
            pending_outproj = []  # eligible outproj group thunks

            def flush_pending():
                nonlocal pending
                if pending is not None:
                    pending[0]()
                    pending[1]()
                    pending_outproj.extend(pending[2])
                    pending = None

            def attn_chunk(q):
                nonlocal pending
                for p in range(2):
                    attnout[p][q] = aop.tile(
                        [128, 512], BF16, tag=f"ao{p}_{q}", name=f"ao{p}_{q}"
                    )
                for h in range(HPC):
                    pav = avps.tile([65, 512], F32, tag="av")
                    prev_av = None
                    for bi, av in enumerate(emit_unit_blocks(h, q, pav)):
                        if prev_av is not None:
                            prev_av()
                        elif pending is not None:
                            flush_pending()
                        prev_av = av
                    after = (
                        [
                            (
                                lambda use_sc=False, q=q, ts=ts, oc=oc: outproj_group(
                                    q, ts, oc, use_sc
                                )
                            )
                            for ts in range(4)
                            for oc in range(2)
                        ]
                        if h == HPC - 1
                        else []
                    )
                    pending = (
                        prev_av,
                        lambda h=h, q=q, pav=pav: normalize(h, q, pav),
                        after,
                    )
                    for _ in range(2):
                        if pending_outproj:
                            pending_outproj.pop(0)()

            proj_chunk(0)
            proj_chunk(1)
            emit_wout_load()
            attn_chunk(0)
            proj_chunk(2)
            attn_chunk(1)
            proj_chunk(3)
            attn_chunk(2)
            attn_chunk(3)
            flush_pending()
            # Tail: scps is free once the last exp has run — alternate the
            # remaining outproj groups across both psum pools so the drain
            # isn't serialized on two mmps bufs.
            for i, th in enumerate(pending_outproj):
                th(use_sc=(i % 2 == 1))

    nc.finalize()
    return nc


def _prep_core_inputs(x, w_qkv, w_out, core):
    import ml_dtypes

    BF = ml_dtypes.bfloat16
    F8 = ml_dtypes.float8_e4m3
    b, hg = core // HG, core % HG
    xT = np.ascontiguousarray(x[b].T)  # [C, T]
    wq = w_qkv[0:C] * np.float32(SQ / np.sqrt(D))
    wk = w_qkv[C : 2 * C] * np.float32(SK)
    wv = w_qkv[2 * C : 3 * C]
    h0 = HPC * hg
    rows = []
    for p in range(2):
        rows.append(wq[64 * (h0 + 2 * p) : 64 * (h0 + 2 * p + 2)])
    for p in range(2):
        rows.append(wk[64 * (h0 + 2 * p) : 64 * (h0 + 2 * p + 2)])
    # [C, 512] column-stacked lhsT, then fp8 DoubleRow layout:
    # [kk(4) x i(2) x p(128), m(4) x c(128)] -> [128, kk, 4(m), 2(i), 128]
    wqk_lhsT = np.concatenate(rows, axis=0).T  # [C, 512]
    wqk8 = (
        np.ascontiguousarray(
            wqk_lhsT.reshape(KK, 2, 128, 4, 128).transpose(2, 0, 3, 1, 4)
        )
        .reshape(128, KK * 1024)
        .astype(F8)
    )
    # x8: [C, T] -> [128, kk*2+i, T] with c = kk*256 + i*128 + p
    x8 = (
        np.ascontiguousarray(xT.reshape(KK, 2, 128, T).transpose(2, 0, 1, 3))
        .reshape(128, 2 * KK, T)
        .astype(F8)
    )
    wv_rhsT = np.ascontiguousarray(wv[64 * h0 : 64 * (h0 + HPC)].T)
    wout_pairs = np.ascontiguousarray(
        w_out[:, 64 * h0 : 64 * (h0 + HPC)].T
    ).reshape(2, 128, C)
    # xT and wv are sent p-major shuffled ([128, CK, ...]) so the kernel can
    # load several contraction k-tiles with one contiguous DMA.
    xTs = np.ascontiguousarray(xT.reshape(CK, 128, T).transpose(1, 0, 2))
    wvs = np.ascontiguousarray(wv_rhsT.reshape(CK, 128, 256).transpose(1, 0, 2))
    return {
        "ones": np.ones((128, 4), dtype=BF),
        "xT": xTs.astype(BF),
        "x8": x8,
        "wqk8": wqk8,
        "wv": wvs.astype(BF),
        "wout": wout_pairs.astype(BF),
    }


def kernel(x, w_qkv, w_out):
    from concourse.bass_utils import run_bass_kernel_spmd

    global _NC
    x = np.asarray(x, dtype=np.float32)
    w_qkv = np.asarray(w_qkv, dtype=np.float32)
    w_out = np.asarray(w_out, dtype=np.float32)

    in_maps = [_prep_core_inputs(x, w_qkv, w_out, c) for c in range(N_CORES)]
    if _NC is None:
        _NC = _build_nc()
    res = run_bass_kernel_spmd(_NC, in_maps, core_ids=list(range(N_CORES)))
    out = np.zeros((B, T, C), dtype=np.float32)
    for c in range(N_CORES):
        out[c // HG] += np.asarray(res.results[c]["y"]).astype(np.float32)
    return out
